# revision 13
# baseline (speedup 1.0000x reference)
"""Trainium2 Bass kernel for nn_CausalBindingMechanism.

Sharding: data-parallel over batch (8 cores, 1 batch element each), no
collectives. Per-core inputs are rotated so the owned batch always sits at
slot-rows 0:16 of the 128-row (batch*slot) token block -> one shared SPMD
program, per-core differences are pure data.

Host-side exact folding:
  - bind q/k projections are dead (softmax over a single key == 1);
    W_bind = bind_v_W @ bind_o_W.
  - P = pos_q @ bcast_q_W + b precomputed (shared across batch), uploaded
    transposed.
  - LN gain/bias folded into consumer Linears (bind_ln -> intv qkv,
    intv_ln -> bcast k/v, bcast_ln -> adapter bottom halves).
  - broadcast-stage out-proj collapsed through the 16-slot bottleneck:
    out = attn_cat @ W2cat with W2cat[(h,s),:] = V2_h[s,:] @ o_W[h-rows,:].

Device: bf16 matmul operands, fp32 PSUM accumulation, fp32 LN/softmax,
slot-attention batched over all 8 batches with a block-diagonal mask.
"""

from contextlib import ExitStack

import numpy as np
import ml_dtypes

import concourse.bass as bass
import concourse.mybir as mybir
import concourse.tile as tile
from concourse import bacc
from concourse._compat import with_exitstack
from concourse.masks import make_identity

F32 = mybir.dt.float32
F32R = mybir.dt.float32r
BF16 = mybir.dt.bfloat16
AF = mybir.ActivationFunctionType
OP = mybir.AluOpType

P = 128
D = 2048
DC = D // P          # 16 d-chunks
L = 512              # seq len
H = 16               # heads
DH = 128             # head dim
S = 16               # slots
NB = 8               # batch
TC = L // P          # 4 token chunks
NT = D // 512        # 4 dout tiles of 512
EPS = 1e-5
ISQ = 1.0 / float(np.sqrt(DH))

_CACHE = {}


def declare_io(nc):
    t = {}

    def din(name, shape, dt):
        t[name] = nc.dram_tensor(name, shape, dt, kind="ExternalInput").ap()

    # shared weights
    for n in ["wbind", "qw", "kw", "vw", "ow", "bckw", "bcvw", "bcow",
              "gwt", "gwb", "a1t", "a1b", "ad2w"]:
        din(n, [D, D], BF16)
    din("pt", [D, L], BF16)
    for n in ["bbind", "qb", "kb", "vb", "ob", "bckb", "bcvb", "bcob",
              "gb", "a1bias", "ad2b", "g1", "b1", "adg", "adb"]:
        din(n, [1, D], F32)
    din("mask", [P, P], F32)
    din("ones16", [S, S], BF16)
    # per-core
    din("slots", [P, D], F32)
    din("ctt", [D, P], BF16)
    din("emat", [P, P], BF16)
    din("dh", [L, D], F32)
    din("dht", [D, L], BF16)
    t["out"] = nc.dram_tensor("out", [L, D], F32, kind="ExternalOutput").ap()
    return t


def build_program():
    nc = bacc.Bacc("TRN2", target_bir_lowering=False, debug=False, num_devices=8)
    t = declare_io(nc)
    with tile.TileContext(nc) as tc:
        _build(tc, t)
    nc.compile()
    return nc


@with_exitstack
def _build(ctx: ExitStack, tc, t):
    nc = tc.nc

    # ---------- pools ----------
    pers = ctx.enter_context(tc.tile_pool(name="pers", bufs=1))
    wpool = ctx.enter_context(tc.tile_pool(name="wpool", bufs=4))
    work = ctx.enter_context(tc.tile_pool(name="work", bufs=2))
    xpool = ctx.enter_context(tc.tile_pool(name="xpool", bufs=3))   # [128, D] f32 shared
    bcst = ctx.enter_context(tc.tile_pool(name="bcst", bufs=3))
    pmm = ctx.enter_context(tc.tile_pool(name="pmm", bufs=4, space="PSUM"))
    ptr = ctx.enter_context(tc.tile_pool(name="ptr", bufs=2, space="PSUM"))
    psm = ctx.enter_context(tc.tile_pool(name="psm", bufs=2, space="PSUM"))

    def bias_bc(name, parts=P):
        bt = bcst.tile([parts, D], F32, tag=f"bc{parts}", name=f"bb_{name}")
        nc.sync.dma_start(out=bt[0:1, :], in_=t[name][0:1, :])
        nc.gpsimd.partition_broadcast(bt, bt[0:1, :])
        return bt

    idb = pers.tile([P, P], BF16)
    make_identity(nc, idb)
    epst = pers.tile([P, 1], F32)
    nc.vector.memset(epst, EPS)

    def transpose_to(dst_ap, src_ap, n_rows=P):
        """dst[128, n_rows] = src[n_rows, 128].T  (bf16 via PE)"""
        pt_ = ptr.tile([P, P], BF16, tag="tr")
        nc.tensor.transpose(pt_[:, :n_rows], src_ap, idb[:n_rows, :n_rows])
        nc.vector.tensor_copy(dst_ap, pt_[:, :n_rows])

    _xn = [0]

    def xtile():
        _xn[0] += 1
        return xpool.tile([P, D], F32, tag="xbig", name=f"xbig{_xn[0]}")

    def layer_norm_stats(x, parts=P):
        xr = x.rearrange("p (n f) -> p n f", f=512)
        st = work.tile([parts, NT, 6], F32, tag="lnst")
        for i in range(NT):
            nc.vector.bn_stats(st[:, i, :], xr[:, i, :])
        mv = work.tile([parts, 2], F32, tag="lnmv")
        nc.vector.bn_aggr(mv, st)
        sd = work.tile([parts, 1], F32, tag="lnsd")
        nc.scalar.activation(sd, mv[:, 1:2], AF.Sqrt, bias=epst[:parts])
        r = work.tile([parts, 1], F32, tag="lnr")
        nc.vector.reciprocal(r, sd)
        return mv[:, 0:1], r

    def stt_add(out_ap, psum_ap, bias_ap):
        nc.vector.tensor_tensor(out=out_ap, in0=psum_ap, in1=bias_ap, op=OP.add)

    # ============================================================
    # Stage 1: bind (content replicated per slot on host -> 128 rows)
    # ============================================================
    ctt_sb = pers.tile([P, DC, P], BF16)
    nc.sync.dma_start(out=ctt_sb, in_=t["ctt"].rearrange("(c p) t -> p c t", p=P))

    slots_sb = xtile()
    nc.sync.dma_start(out=slots_sb, in_=t["slots"])
    bbb = bias_bc("bbind")
    x1 = xtile()
    for dt_i in range(NT):
        sl = slice(dt_i * 512, (dt_i + 1) * 512)
        pd = pmm.tile([P, 512], F32, tag="mm")
        for c in range(DC):
            wt = wpool.tile([P, 512], BF16, tag="w512")
            nc.sync.dma_start(out=wt, in_=t["wbind"][c * P:(c + 1) * P, sl])
            nc.tensor.matmul(pd, ctt_sb[:, c, :], wt, start=(c == 0), stop=(c == DC - 1))
        stt_add(x1[:, sl], pd, bbb[:, sl])
        nc.vector.tensor_tensor(out=x1[:, sl], in0=x1[:, sl], in1=slots_sb[:, sl],
                                op=OP.add)

    m1, r1 = layer_norm_stats(x1)
    xhat1 = xtile()
    nc.vector.tensor_scalar(out=xhat1, in0=x1, scalar1=m1, scalar2=r1,
                            op0=OP.subtract, op1=OP.mult)
    xhat1b = work.tile([P, D], BF16, tag="xb1")
    nc.vector.tensor_copy(xhat1b, xhat1)
    g1b = bias_bc("g1")
    b1b = bias_bc("b1")
    bound1 = xtile()
    nc.vector.tensor_tensor(out=bound1, in0=xhat1, in1=g1b, op=OP.mult)
    nc.vector.tensor_tensor(out=bound1, in0=bound1, in1=b1b, op=OP.add)

    # ============================================================
    # Stage 2: intv (all batches, block-diagonal attention)
    # ============================================================
    def gemm_tokmajor(lhsT_sb, w_name, bias_tile, out_tile):
        for dt_i in range(NT):
            sl = slice(dt_i * 512, (dt_i + 1) * 512)
            pq = pmm.tile([P, 512], F32, tag="mm")
            for c in range(DC):
                wt = wpool.tile([P, 512], BF16, tag="w512")
                nc.sync.dma_start(out=wt, in_=t[w_name][c * P:(c + 1) * P, sl])
                nc.tensor.matmul(pq, lhsT_sb[:, c, :], wt,
                                 start=(c == 0), stop=(c == DC - 1))
            stt_add(out_tile[:, sl], pq, bias_tile[:, sl])

    with tc.tile_pool(name="st2", bufs=1) as st2:
        xhat1T = st2.tile([P, DC, P], BF16)
        for c in range(DC):
            transpose_to(xhat1T[:, c, :], xhat1b[:, c * P:(c + 1) * P])

        q_sb = st2.tile([P, D], BF16, tag="qsb")
        k_sb = st2.tile([P, D], BF16, tag="ksb")
        v_sb = st2.tile([P, D], BF16, tag="vsb")
        gemm_tokmajor(xhat1T, "qw", bias_bc("qb"), q_sb)
        gemm_tokmajor(xhat1T, "kw", bias_bc("kb"), k_sb)
        gemm_tokmajor(xhat1T, "vw", bias_bc("vb"), v_sb)

        qT = st2.tile([P, H, P], BF16, tag="qT")
        kT = st2.tile([P, H, P], BF16, tag="kT")
        for h in range(H):
            transpose_to(qT[:, h, :], q_sb[:, h * DH:(h + 1) * DH])
            transpose_to(kT[:, h, :], k_sb[:, h * DH:(h + 1) * DH])

        mask_sb = st2.tile([P, P], F32)
        nc.sync.dma_start(out=mask_sb, in_=t["mask"])

        zT = st2.tile([P, DC, P], BF16)
        for h in range(H):
            ps = psm.tile([P, 512], F32, tag="small")
            nc.tensor.matmul(ps[:, :P], qT[:, h, :], kT[:, h, :], start=True, stop=True)
            ex = work.tile([P, P], F32, tag="ex")
            nc.scalar.activation(ex, ps[:, :P], AF.Exp, scale=ISQ)
            exm = work.tile([P, P], F32, tag="exm")
            nc.vector.tensor_tensor(out=exm, in0=ex, in1=mask_sb, op=OP.mult)
            den = work.tile([P, 1], F32, tag="den")
            nc.vector.reduce_sum(den, exm, axis=mybir.AxisListType.X)
            rden = work.tile([P, 1], F32, tag="rden")
            nc.vector.reciprocal(rden, den)
            attn = work.tile([P, P], BF16, tag="attnb")
            nc.vector.tensor_scalar(out=attn, in0=exm, scalar1=rden, scalar2=None,
                                    op0=OP.mult)
            attnT = work.tile([P, P], BF16, tag="attnT")
            transpose_to(attnT, attn)
            pz = psm.tile([P, 512], F32, tag="small")
            nc.tensor.matmul(pz[:, :P], v_sb[:, h * DH:(h + 1) * DH], attnT,
                             start=True, stop=True)
            nc.vector.tensor_copy(zT[:, h, :], pz[:, :P])

        msg = st2.tile([P, D], BF16, tag="msg")
        gemm_tokmajor(zT, "ow", bias_bc("ob"), msg)

        emat_sb = st2.tile([P, P], BF16, tag="emat")
        nc.sync.dma_start(out=emat_sb, in_=t["emat"])
        x2 = xtile()
        for dt_i in range(NT):
            sl = slice(dt_i * 512, (dt_i + 1) * 512)
            pe = pmm.tile([P, 512], F32, tag="mm")
            nc.tensor.matmul(pe, emat_sb, msg[:, sl], start=True, stop=True)
            nc.vector.tensor_tensor(out=x2[:, sl], in0=pe, in1=bound1[:, sl], op=OP.add)

        m2, r2 = layer_norm_stats(x2)
        xhat2b = work.tile([P, D], BF16, tag="xb1")
        nc.vector.tensor_scalar(out=xhat2b, in0=x2, scalar1=m2, scalar2=r2,
                                op0=OP.subtract, op1=OP.mult)

    # ============================================================
    # Stage 3: broadcast attention (own batch = slot rows 0:16)
    # ============================================================
    bcT = pers.tile([P, DC, L], BF16)
    with tc.tile_pool(name="st3", bufs=1) as st3:
        xhat2T = st3.tile([P, DC, P], BF16)
        for c in range(DC):
            transpose_to(xhat2T[:, c, :], xhat2b[:, c * P:(c + 1) * P])

        k2 = st3.tile([P, D], BF16, tag="k2")
        v2 = st3.tile([P, D], BF16, tag="v2")
        gemm_tokmajor(xhat2T, "bckw", bias_bc("bckb"), k2)
        gemm_tokmajor(xhat2T, "bcvw", bias_bc("bcvb"), v2)

        k2T = st3.tile([P, H, S], BF16, tag="k2T")
        v2T = st3.tile([P, H, S], BF16, tag="v2T")
        for h in range(H):
            transpose_to(k2T[:, h, :], k2[0:S, h * DH:(h + 1) * DH], n_rows=S)
            transpose_to(v2T[:, h, :], v2[0:S, h * DH:(h + 1) * DH], n_rows=S)

        w2cat = [st3.tile([P, D], BF16, tag=f"w2c{i}", name=f"w2c{i}") for i in range(2)]
        for h in range(H):
            for dt_i in range(NT):
                sl = slice(dt_i * 512, (dt_i + 1) * 512)
                pw = psm.tile([S, 512], F32, tag="small")
                wt = wpool.tile([P, 512], BF16, tag="w512")
                nc.sync.dma_start(out=wt, in_=t["bcow"][h * DH:(h + 1) * DH, sl])
                nc.tensor.matmul(pw, v2T[:, h, :], wt, start=True, stop=True)
                pw_sb = work.tile([S, 512], BF16, tag="pwsb")
                nc.vector.tensor_copy(pw_sb, pw)
                nc.sync.dma_start(
                    out=w2cat[h // 8][(h % 8) * S:(h % 8 + 1) * S, sl], in_=pw_sb)

        pt_sb = st3.tile([P, DC, L], BF16)
        nc.sync.dma_start(out=pt_sb, in_=t["pt"].rearrange("(c p) t -> p c t", p=P))
        ones_sb = st3.tile([S, S], BF16, tag="ones")
        nc.sync.dma_start(out=ones_sb, in_=t["ones16"])

        acat = [st3.tile([P, L], BF16, tag=f"ac{i}", name=f"ac{i}") for i in range(2)]
        for h in range(H):
            psc = psm.tile([S, L], F32, tag="small")
            nc.tensor.matmul(psc, k2T[:, h, :], pt_sb[:, h, :], start=True, stop=True)
            exb = work.tile([S, L], BF16, tag="exb")
            nc.scalar.activation(exb, psc, AF.Exp, scale=ISQ)
            pden = psm.tile([S, L], F32, tag="small")
            nc.tensor.matmul(pden, ones_sb, exb, start=True, stop=True)
            rden = work.tile([S, L], F32, tag="rdenb")
            nc.vector.reciprocal(rden, pden)
            at = work.tile([S, L], BF16, tag="atb")
            nc.vector.tensor_tensor(out=at, in0=exb, in1=rden, op=OP.mult)
            nc.sync.dma_start(out=acat[h // 8][(h % 8) * S:(h % 8 + 1) * S, :], in_=at)

        obb = bias_bc("bcob")
        for tci in range(TC):
            tsl = slice(tci * P, (tci + 1) * P)
            xbc = xtile()
            for dt_i in range(NT):
                sl = slice(dt_i * 512, (dt_i + 1) * 512)
                pb = pmm.tile([P, 512], F32, tag="mm")
                for kc in range(2):
                    nc.tensor.matmul(pb, acat[kc][:, tsl], w2cat[kc][:, sl],
                                     start=(kc == 0), stop=(kc == 1))
                stt_add(xbc[:, sl], pb, obb[:, sl])
            mb, rb = layer_norm_stats(xbc)
            xbcb = work.tile([P, D], BF16, tag="xb1")
            nc.vector.tensor_scalar(out=xbcb, in0=xbc, scalar1=mb, scalar2=rb,
                                    op0=OP.subtract, op1=OP.mult)
            for c in range(DC):
                transpose_to(bcT[:, c, tsl], xbcb[:, c * P:(c + 1) * P])

    # ============================================================
    # Stage 4: gated adapter (weights streamed once; tci innermost)
    # ============================================================
    with tc.tile_pool(name="st4", bufs=1) as st4:
        dht_sb = st4.tile([P, DC, L], BF16)
        nc.sync.dma_start(out=dht_sb, in_=t["dht"].rearrange("(c p) t -> p c t", p=P))

        def gemm2_col(wtop, wbot, dt_i, psums):
            """accumulate [dh | bcast] @ W[:, dt_i] into psums[tci], 32 chunks"""
            sl = slice(dt_i * 512, (dt_i + 1) * 512)
            for c in range(DC):
                wt = wpool.tile([P, 512], BF16, tag="w512")
                nc.sync.dma_start(out=wt, in_=t[wtop][c * P:(c + 1) * P, sl])
                for tci in range(TC):
                    nc.tensor.matmul(psums[tci], dht_sb[:, c, tci * P:(tci + 1) * P],
                                     wt, start=(c == 0), stop=False)
            for c in range(DC):
                wt = wpool.tile([P, 512], BF16, tag="w512")
                nc.sync.dma_start(out=wt, in_=t[wbot][c * P:(c + 1) * P, sl])
                for tci in range(TC):
                    nc.tensor.matmul(psums[tci], bcT[:, c, tci * P:(tci + 1) * P],
                                     wt, start=False, stop=(c == DC - 1))

        gbb = bias_bc("gb")
        gate = [st4.tile([P, D], BF16, tag=f"gate{i}", name=f"gate{i}")
                for i in range(TC)]
        for dt_i in range(NT):
            sl = slice(dt_i * 512, (dt_i + 1) * 512)
            psums = [pmm.tile([P, 512], F32, tag="mm", name=f"pg{dt_i}_{i}")
                     for i in range(TC)]
            gemm2_col("gwt", "gwb", dt_i, psums)
            for tci in range(TC):
                gr = work.tile([P, 512], F32, tag="gr")
                stt_add(gr, psums[tci], gbb[:, sl])
                nc.scalar.activation(gate[tci][:, sl], gr, AF.Sigmoid)

        a1bb = bias_bc("a1bias")
        hfull = [st4.tile([P, D], F32, tag=f"h{i}", name=f"hfull{i}")
                 for i in range(TC)]
        for dt_i in range(NT):
            sl = slice(dt_i * 512, (dt_i + 1) * 512)
            psums = [pmm.tile([P, 512], F32, tag="mm", name=f"ph{dt_i}_{i}")
                     for i in range(TC)]
            gemm2_col("a1t", "a1b", dt_i, psums)
            for tci in range(TC):
                stt_add(hfull[tci][:, sl], psums[tci], a1bb[:, sl])

        adgb = bias_bc("adg")
        adbb = bias_bc("adb")
        ghT = st4.tile([P, DC, L], BF16)
        for tci in range(TC):
            tsl = slice(tci * P, (tci + 1) * P)
            mh, rh = layer_norm_stats(hfull[tci])
            xh = xtile()
            nc.vector.tensor_scalar(out=xh, in0=hfull[tci], scalar1=mh, scalar2=rh,
                                    op0=OP.subtract, op1=OP.mult)
            nc.vector.tensor_tensor(out=xh, in0=xh, in1=adgb, op=OP.mult)
            nc.vector.tensor_tensor(out=xh, in0=xh, in1=adbb, op=OP.add)
            ghb = work.tile([P, D], BF16, tag="xb1")
            nc.scalar.activation(ghb, xh, AF.Gelu)
            for c in range(DC):
                transpose_to(ghT[:, c, tsl], ghb[:, c * P:(c + 1) * P])

        ad2bb = bias_bc("ad2b")
        out_r = t["out"].rearrange("(tc p) d -> p tc d", p=P)
        dh_r = t["dh"].rearrange("(tc p) d -> p tc d", p=P)
        for dt_i in range(NT):
            sl = slice(dt_i * 512, (dt_i + 1) * 512)
            psums = [pmm.tile([P, 512], F32, tag="mm", name=f"pa{dt_i}_{i}")
                     for i in range(TC)]
            for c in range(DC):
                wt = wpool.tile([P, 512], BF16, tag="w512")
                nc.sync.dma_start(out=wt, in_=t["ad2w"][c * P:(c + 1) * P, sl])
                for tci in range(TC):
                    nc.tensor.matmul(psums[tci], ghT[:, c, tci * P:(tci + 1) * P],
                                     wt, start=(c == 0), stop=(c == DC - 1))
            for tci in range(TC):
                # out = dh + gate * (pa + ad2b - dh)
                dhc = work.tile([P, 512], F32, tag="dhc")
                nc.sync.dma_start(out=dhc, in_=dh_r[:, tci, sl])
                d1 = work.tile([P, 512], F32, tag="d1")
                stt_add(d1, psums[tci], ad2bb[:, sl])
                nc.vector.tensor_tensor(out=d1, in0=d1, in1=dhc, op=OP.subtract)
                nc.vector.tensor_tensor(out=d1, in0=d1, in1=gate[tci][:, sl], op=OP.mult)
                od = work.tile([P, 512], F32, tag="od")
                nc.vector.tensor_tensor(out=od, in0=d1, in1=dhc, op=OP.add)
                nc.sync.dma_start(out=out_r[:, tci, sl], in_=od)


# ================= host side =================

def _prep(structural_slots, content_repr, edge_weights, decoder_hidden, params):
    f32 = np.float32
    bf = ml_dtypes.bfloat16
    p = {k: np.asarray(v, f32) for k, v in params.items()}

    W_bind = (p["bind_v_W"] @ p["bind_o_W"]).astype(f32)
    b_bind = (p["bind_v_b"] @ p["bind_o_W"] + p["bind_o_b"]).astype(f32)
    Pq = (p["pos_q"][0, :L] @ p["bcast_q_W"] + p["bcast_q_b"]).astype(f32)

    g1, b1 = p["bind_ln_g"], p["bind_ln_b"]
    g2, b2 = p["intv_ln_g"], p["intv_ln_b"]
    g3, b3 = p["bcast_ln_g"], p["bcast_ln_b"]

    shared = {
        "wbind": W_bind.astype(bf),
        "qw": (g1[:, None] * p["intv_q_W"]).astype(bf),
        "kw": (g1[:, None] * p["intv_k_W"]).astype(bf),
        "vw": (g1[:, None] * p["intv_v_W"]).astype(bf),
        "ow": p["intv_o_W"].astype(bf),
        "bckw": (g2[:, None] * p["bcast_k_W"]).astype(bf),
        "bcvw": (g2[:, None] * p["bcast_v_W"]).astype(bf),
        "bcow": p["bcast_o_W"].astype(bf),
        "pt": np.ascontiguousarray(Pq.T).astype(bf),
        "gwt": p["gate_W"][:D].astype(bf),
        "gwb": (g3[:, None] * p["gate_W"][D:]).astype(bf),
        "a1t": p["ad1_W"][:D].astype(bf),
        "a1b": (g3[:, None] * p["ad1_W"][D:]).astype(bf),
        "ad2w": p["ad2_W"].astype(bf),
        "bbind": b_bind.reshape(1, D),
        "qb": (b1 @ p["intv_q_W"] + p["intv_q_b"]).reshape(1, D).astype(f32),
        "kb": (b1 @ p["intv_k_W"] + p["intv_k_b"]).reshape(1, D).astype(f32),
        "vb": (b1 @ p["intv_v_W"] + p["intv_v_b"]).reshape(1, D).astype(f32),
        "ob": p["intv_o_b"].reshape(1, D).astype(f32),
        "bckb": (b2 @ p["bcast_k_W"] + p["bcast_k_b"]).reshape(1, D).astype(f32),
        "bcvb": (b2 @ p["bcast_v_W"] + p["bcast_v_b"]).reshape(1, D).astype(f32),
        "bcob": p["bcast_o_b"].reshape(1, D).astype(f32),
        "gb": (p["gate_b"] + b3 @ p["gate_W"][D:]).reshape(1, D).astype(f32),
        "a1bias": (p["ad1_b"] + b3 @ p["ad1_W"][D:]).reshape(1, D).astype(f32),
        "ad2b": p["ad2_b"].reshape(1, D).astype(f32),
        "g1": g1.reshape(1, D).astype(f32),
        "b1": b1.reshape(1, D).astype(f32),
        "adg": p["ad_ln_g"].reshape(1, D).astype(f32),
        "adb": p["ad_ln_b"].reshape(1, D).astype(f32),
        "mask": np.kron(np.eye(NB, dtype=f32), np.ones((S, S), f32)),
        "ones16": np.ones((S, S), ml_dtypes.bfloat16),
    }

    ss = np.asarray(structural_slots, f32)
    cr = np.asarray(content_repr, f32)
    ew = np.asarray(edge_weights, f32)
    dhf = np.asarray(decoder_hidden, f32)

    in_maps = []
    for c in range(NB):
        order = [(c + i) % NB for i in range(NB)]
        emat = np.zeros((P, P), f32)
        for i, b in enumerate(order):
            emat[i * S:(i + 1) * S, i * S:(i + 1) * S] = ew[b]
        m = dict(shared)
        m.update({
            "slots": np.ascontiguousarray(ss[order].reshape(P, D)),
            "ctt": np.ascontiguousarray(np.repeat(cr[order], S, axis=0).T).astype(bf),
            "emat": emat.astype(bf),
            "dh": np.ascontiguousarray(dhf[c]),
            "dht": np.ascontiguousarray(dhf[c].T).astype(bf),
        })
        in_maps.append(m)
    return in_maps


def kernel(structural_slots, content_repr, edge_weights, decoder_hidden, params):
    from concourse.bass_utils import run_bass_kernel_spmd
    if "nc" not in _CACHE:
        _CACHE["nc"] = build_program()
    nc = _CACHE["nc"]
    in_maps = _prep(structural_slots, content_repr, edge_weights,
                    decoder_hidden, params)
    res = run_bass_kernel_spmd(nc, in_maps, core_ids=list(range(NB)))
    out = np.stack([res.results[c]["out"] for c in range(NB)], axis=0)
    return out.astype(np.float32)
